# revision 7
# baseline (speedup 1.0000x reference)
"""DenseCorr2d full kernel for 8 Trainium2 NeuronCores.

Reference computation (per example b):
  corr[(cm*16+ct), y, x] = sum_{dy,dx} tm_edgepad[cm, y+dy, x+dx] * tp[ct, dy, dx]
  out[co, y, x] = bias[co] + sum_{ci,ky,kx} W[co, ci, ky, kx] * corr_zpad[ci, y+ky-1, x+kx-1]

Sharding: data-parallel over batch; core i computes example i entirely.

Stage A runs the dense correlation as 16 PSUM-accumulating matmuls per
spatial tile: the moving operand holds 8 cm-channels x 16 baked x-shifts of
the edge-padded image on the 128 partitions, the stationary is a
block-diagonal arrangement of the template row tp[:, dy, :]; accumulation
over dy happens in PSUM.  corr is kept resident in SBUF zero-padded to
130x130 per 128-channel chunk.

Stage B runs the 3x3 'same' merge conv as 18 PSUM-accumulating matmuls
(9 taps x 2 input-channel chunks) with the tap shift expressed as a free-dim
offset into the padded corr, bias fused into the ScalarE PSUM->SBUF copy.

Matmuls use float32r (full PE rate at N>=256, near-fp32 precision).
"""

from contextlib import ExitStack

import numpy as np

import bass_rust
import concourse.bass as bass
import concourse.tile as tile
from concourse import bacc, mybir
from concourse.bass_utils import run_bass_kernel_spmd

F32 = mybir.dt.float32
F32R = mybir.dt.float32r

N_CORES = 8
# Problem shapes (hardcoded per contract).
B, CT, HT, WT = 8, 16, 16, 16
CM, HM, WM = 16, 128, 128
COUT, K = 64, 3
HP = HM + HT - 1  # 143 padded image rows/cols
ROWS_BLK = 32  # output rows produced per Mblk load
WIN = ROWS_BLK + WT - 1  # 47 input rows needed per block
BAND = 3  # output rows per stage-B matmul band

_CACHE: dict = {}


def _r(ap):
    return ap.bitcast(F32R)


def _emit(ctx: ExitStack, tc, nc, tmp, sa, wst, bia, out):
    const = ctx.enter_context(tc.tile_pool(name="const", bufs=1))
    corrp = ctx.enter_context(tc.tile_pool(name="corrp", bufs=1))

    sa_sb = const.tile([128, 16, 128], F32R, name="sa_sb")
    nc.sync.dma_start(out=sa_sb[:], in_=sa.ap())
    w_sb = const.tile([128, 18, COUT], F32R, name="w_sb")
    nc.sync.dma_start(out=w_sb[:], in_=wst.ap())
    b_sb = const.tile([COUT, 1], F32, name="b_sb")
    nc.sync.dma_start(out=b_sb[:], in_=bia.ap())

    # corr, zero-padded: 2 chunks of 130x130 rows/cols, chunk c = channels
    # [c*128, (c+1)*128) on partitions.
    # 2 elements of tail slack: the last band's kx-shifted windows read (and
    # discard) up to 2 elements past the padded grid.
    corr_sb = corrp.tile([128, 2 * 130 * 130 + 2], F32R, name="corr_sb")
    corr_flat = corr_sb[:]
    corr = corr_sb[:, : 2 * 130 * 130].rearrange("p (a b) -> p a b", a=2 * 130)
    # Zero the padding borders by DMA from a host-supplied zeros tensor
    # (memset can't emit float32r, and the fp32r provenance verifier
    # rejects fp32-written bytes feeding fp32r matmuls).
    zz = _CACHE["zz_handle"]
    nc.sync.dma_start(out=corr_sb[:, 2 * 130 * 130 :], in_=zz.ap()[:, :2])
    for c in range(2):
        nc.sync.dma_start(out=corr[:, c * 130, :], in_=zz.ap()[:, :130])
        nc.sync.dma_start(out=corr[:, c * 130 + 129, :], in_=zz.ap()[:, :130])
        nc.sync.dma_start(
            out=corr[:, c * 130 : (c + 1) * 130, 0], in_=zz.ap()[:, :130]
        )
        nc.sync.dma_start(
            out=corr[:, c * 130 : (c + 1) * 130, 129], in_=zz.ap()[:, :130]
        )

    # ---- Stage A ----
    with (
        tc.tile_pool(name="mblk", bufs=2) as mpool,
        tc.tile_pool(name="psA", bufs=8, space="PSUM") as psA,
    ):
        for h in range(2):  # cm halves (8 channels each)
            for blk in range(HM // ROWS_BLK):
                r0 = ROWS_BLK * blk
                mt = mpool.tile([128, WIN, WM], F32R, name="mt", tag="mt")
                # partition (g, j) <- tm_pad[8h+g, r0+y, x+j]; one DMA per g
                # (the DMA AP balancer tops out at 3 dims).
                for g in range(8):
                    src = tmp.ap()[8 * h + g, r0 : r0 + WIN, :WM]
                    src.ap = bass_rust.VecI64Pair([[1, 16], [HP, WIN], [1, WM]])
                    nc.sync.dma_start(
                        out=mt[g * 16 : (g + 1) * 16, :, :], in_=src
                    )
                mflat = mt[:].rearrange("p a b -> p (a b)")
                pts = [
                    psA.tile([128, 512], F32, name=f"pA{sp}", tag="pA")
                    for sp in range(8)
                ]
                for dy in range(WT):
                    for sp in range(8):
                        o = (4 * sp + dy) * WM
                        nc.tensor.matmul(
                            pts[sp][:],
                            sa_sb[:, dy, :],
                            mflat[:, o : o + 512],
                            start=(dy == 0),
                            stop=(dy == WT - 1),
                        )
                for sp in range(8):
                    rr = h * 130 + r0 + 4 * sp + 1
                    nc.vector.tensor_copy(
                        corr[:, rr : rr + 4, 1:129],
                        pts[sp][:].rearrange("p (a b) -> p a b", a=4),
                    )

    # ---- Stage B ----
    with (
        tc.tile_pool(name="psB", bufs=4, space="PSUM") as psB,
        tc.tile_pool(name="outp", bufs=3) as outp,
    ):
        n_bands = (HM + BAND - 1) // BAND
        for band in range(n_bands):
            y0 = BAND * band
            rows = min(BAND, HM - y0)
            n = rows * 130
            pb = psB.tile([COUT, BAND, 130], F32, name="pb", tag="pb")
            pbf = pb[:, :rows, :].rearrange("p a b -> p (a b)")
            for c in range(2):
                for s in range(9):
                    ky, kx = divmod(s, 3)
                    off = (c * 130 + y0 + ky) * 130 + kx
                    nc.tensor.matmul(
                        pbf,
                        w_sb[:, c * 9 + s, :],
                        corr_flat[:, off : off + n],
                        start=(c == 0 and s == 0),
                        stop=(c == 1 and s == 8),
                    )
            ot = outp.tile([COUT, BAND, WM], F32, name="ot", tag="ot")
            nc.scalar.activation(
                ot[:, :rows, :],
                pb[:, :rows, 0:128],
                mybir.ActivationFunctionType.Identity,
                bias=b_sb[:, 0:1],
            )
            nc.sync.dma_start(
                out=out.ap()[:, y0 : y0 + rows, :], in_=ot[:, :rows, :]
            )


def _build():
    nc = bacc.Bacc("TRN2", target_bir_lowering=False, debug=False)
    tmp = nc.dram_tensor("tmp", [CM, HP, HP], F32R, kind="ExternalInput")
    sa = nc.dram_tensor("sa", [128, 16, 128], F32R, kind="ExternalInput")
    wst = nc.dram_tensor("wst", [128, 18, COUT], F32R, kind="ExternalInput")
    bia = nc.dram_tensor("bias", [COUT, 1], F32, kind="ExternalInput")
    _CACHE["zz_handle"] = nc.dram_tensor(
        "zz", [128, 130], F32R, kind="ExternalInput"
    )
    out = nc.dram_tensor("out", [COUT, HM, WM], F32, kind="ExternalOutput")
    with tile.TileContext(nc) as tc, ExitStack() as ctx:
        _emit(ctx, tc, nc, tmp, sa, wst, bia, out)
    nc.compile()
    return nc


def _get_nc():
    if "nc" not in _CACHE:
        _CACHE["nc"] = _build()
    return _CACHE["nc"]


def _host_prep(template, tomatch, W, b):
    template = np.ascontiguousarray(template, dtype=np.float32)
    tomatch = np.ascontiguousarray(tomatch, dtype=np.float32)
    W = np.ascontiguousarray(W, dtype=np.float32)
    b = np.ascontiguousarray(b, dtype=np.float32)

    tm_pad = np.pad(
        tomatch, ((0, 0), (0, 0), (0, HT - 1), (0, WT - 1)), mode="edge"
    )  # [B, CM, 143, 143]

    # sa[b, g*16+j, dy, g*16+ct] = template[b, ct, dy, j]
    sa = np.zeros((B, 128, 16, 128), np.float32)
    tpT = template.transpose(0, 3, 2, 1)  # [b, dx, dy, ct]
    for g in range(8):
        sa[:, g * 16 : g * 16 + 16, :, g * 16 : g * 16 + 16] = tpT

    # wst[k, c*9 + ky*3 + kx, co] = W[co, c*128+k, ky, kx]
    wst = np.ascontiguousarray(
        W.reshape(COUT, 2, 128, K, K).transpose(2, 1, 3, 4, 0).reshape(128, 18, COUT)
    )
    bias = np.ascontiguousarray(b.reshape(COUT, 1))
    zz = np.zeros((128, 130), np.float32)
    return tm_pad, sa, wst, bias, zz


def kernel(template, tomatch, W, b):
    tm_pad, sa, wst, bias, zz = _host_prep(template, tomatch, W, b)
    nc = _get_nc()
    in_maps = [
        {"tmp": tm_pad[i], "sa": sa[i], "wst": wst, "bias": bias, "zz": zz}
        for i in range(N_CORES)
    ]
    res = run_bass_kernel_spmd(nc, in_maps, list(range(N_CORES)))
    return np.stack([res.results[i]["out"] for i in range(N_CORES)])
